# revision 1
# baseline (speedup 1.0000x reference)
"""Centroid triplet loss on 8 Trainium2 NeuronCores (Bass/Tile).

Data-parallel over the batch: each of the 8 cores gets 8192 of the 65536
samples.  Per-class embedding sums and counts are all-reduced to form global
centroids; each core then computes its local triplet terms and a final
all-reduce produces the scalar loss.

Math restructure (equivalent to the reference):
    term_i = relu(margin + e_hat_i . (cent[nearest[l_i]] - cent[l_i]))
    loss   = sum_i w_{l_i} * term_i / n_present,   w_c = 1/max(count_c, 1)
Since relu(w*x) = w*relu(x) for w > 0, a sample's weighted term is
    relu(b_{l_i} + r_i * (e_i . u_{l_i}))
with u_c = w_c*(cent_near_c - cent_c), b_c = w_c*margin, r_i = 1/||e_i||.
So embeddings stay raw in SBUF; the one-hot used for the class-sum matmul is
scaled by r_i, and pass 2 gathers (u_c, b_c) rows per sample by label and
fuses the dot product via tensor_tensor_reduce.
"""

import sys

for _p in ("/opt/trn_rl_repo",):
    if _p not in sys.path:
        sys.path.insert(0, _p)

from contextlib import ExitStack

import numpy as np

from concourse import bacc, bass, mybir, tile
from concourse.bass_utils import run_bass_kernel_spmd
from concourse.masks import make_identity

F32 = mybir.dt.float32
BF16 = mybir.dt.bfloat16
I32 = mybir.dt.int32
I16 = mybir.dt.int16
ALU = mybir.AluOpType
ACTF = mybir.ActivationFunctionType

N_CORES = 8
B_FULL = 65536
D = 512
C = 256
MARGIN = 0.3
EPS = 1e-12

P = 128                      # SBUF partitions
B_LOC = B_FULL // N_CORES    # 8192 samples per core
T = B_LOC // P               # 64 sample tiles of 128
LOAD_CHUNK = 8               # tiles per embedding-load DMA (2 MiB each)
TBL_B = 640                  # bf16 table row: k*u[0:512], b_hi, b_lo, u2_hi, u2_lo, pad
                             # (1280B, multiple of 256B for dma_gather)
GCHUNK = 1024                # indices per dma_gather call (8 sample tiles)
NEG = -1e30
KAPPA = 256.0                # scale for the difference-of-squares dot trick


def _build():
    nc = bacc.Bacc(
        "TRN2",
        target_bir_lowering=False,
        debug=False,
        enable_asserts=False,
        num_devices=N_CORES,
    )

    emb = nc.dram_tensor("emb", [B_LOC, D], F32, kind="ExternalInput")
    lab = nc.dram_tensor("lab", [P, T], I32, kind="ExternalInput")
    # labels in dma_gather's wrapped-int16 layout: idx i lives at
    # [i % 16, i // 16], replicated into all eight 16-partition groups
    lab16 = nc.dram_tensor("lab16", [P, B_LOC // 16], I16, kind="ExternalInput")
    loss_out = nc.dram_tensor("loss", [1, 1], F32, kind="ExternalOutput")

    # Internal HBM scratch.  AR1 buffer: rows 0:256 per-class sums, row 256
    # carries the per-class counts in its first 256 columns.
    ar1_in = nc.dram_tensor("ar1_in", [C + 1, D], F32)
    ar1_out = nc.dram_tensor("ar1_out", [C + 1, D], F32, addr_space="Shared")
    table = nc.dram_tensor("tbl", [C, TBL_B], BF16)
    ar2_in = nc.dram_tensor("ar2_in", [1, 8], F32)
    ar2_out = nc.dram_tensor("ar2_out", [1, 8], F32, addr_space="Shared")

    groups = [list(range(N_CORES))]

    with tile.TileContext(nc) as tc, ExitStack() as ctx:
        const = ctx.enter_context(tc.tile_pool(name="const", bufs=1))
        big = ctx.enter_context(tc.tile_pool(name="big", bufs=1))
        work = ctx.enter_context(tc.tile_pool(name="work", bufs=3))
        sq = ctx.enter_context(tc.tile_pool(name="sq", bufs=2))
        gat = ctx.enter_context(tc.tile_pool(name="gat", bufs=4))
        mid = ctx.enter_context(tc.tile_pool(name="mid", bufs=1))
        psacc = ctx.enter_context(tc.tile_pool(name="psacc", bufs=1, space="PSUM"))
        psmid = ctx.enter_context(tc.tile_pool(name="psmid", bufs=3, space="PSUM"))

        # ---- constants -------------------------------------------------
        ident = const.tile([P, P], F32)
        make_identity(nc, ident[:])
        iota_row = const.tile([P, C], BF16)
        nc.gpsimd.iota(
            iota_row[:], pattern=[[1, C]], base=0, channel_multiplier=0,
            allow_small_or_imprecise_dtypes=True,
        )
        ones_col = const.tile([P, 1], F32)
        nc.gpsimd.memset(ones_col[:], 1.0)
        ones_col_bf = const.tile([P, 1], BF16)
        nc.gpsimd.memset(ones_col_bf[:], 1.0)
        ones_row = const.tile([1, P], F32)
        nc.gpsimd.memset(ones_row[:], 1.0)

        lab_sb = const.tile([P, T], I32)
        nc.sync.dma_start(out=lab_sb[:], in_=lab.ap())
        lab_f = const.tile([P, T], BF16)
        nc.vector.tensor_copy(out=lab_f[:], in_=lab_sb[:])
        lab16_sb = const.tile([P, B_LOC // 16], I16)
        nc.sync.dma_start(out=lab16_sb[:], in_=lab16.ap())

        # ---- pass 1: load embeddings, norms, class sums/counts ---------
        e_chunks = []
        emb_v = emb.ap().rearrange("(t p) d -> p t d", p=P)
        for ci in range(T // LOAD_CHUNK):
            # bf16 residency: halves SBUF and lets the class-sum matmuls run
            # single-pass bf16 instead of fp32 HI/LO pairs (cast in the DMA,
            # SWDGE-only feature)
            ec = big.tile([P, LOAD_CHUNK, D], BF16, tag=f"e{ci}")
            e_chunks.append(ec)
            sl = slice(ci * LOAD_CHUNK, (ci + 1) * LOAD_CHUNK)
            nc.gpsimd.dma_start(out=ec[:], in_=emb_v[:, sl, :])

        norm2 = const.tile([P, T], F32)
        norm = const.tile([P, T], F32)
        r_all = const.tile([P, T], F32)
        r_bf = const.tile([P, T], BF16)

        sums_ps0 = psacc.tile([P, D], F32, tag="sums0")
        sums_ps1 = psacc.tile([P, D], F32, tag="sums1")
        cnt_ps = psacc.tile([1, C], F32, tag="cnt")

        def e_tile(t):
            return e_chunks[t // LOAD_CHUNK][:, t % LOAD_CHUNK, :]

        for ci in range(T // LOAD_CHUNK):
            csl = slice(ci * LOAD_CHUNK, (ci + 1) * LOAD_CHUNK)
            for j in range(LOAD_CHUNK):
                t = ci * LOAD_CHUNK + j
                sq_t = sq.tile([P, D], F32, tag="sq")
                # tensor_tensor_reduce is broken on this runtime (kills the
                # exec unit) — use ACT Square with free-dim accumulation.
                nc.scalar.activation(
                    sq_t[:], e_tile(t), ACTF.Square,
                    accum_out=norm2[:, t : t + 1],
                )
            # batched per-chunk norm -> r (cheaper than per-tile column ops)
            nc.scalar.activation(norm[:, csl], norm2[:, csl], ACTF.Sqrt)
            nc.vector.reciprocal(r_all[:, csl], norm[:, csl])
            nc.vector.tensor_copy(out=r_bf[:, csl], in_=r_all[:, csl])

            for j in range(LOAD_CHUNK):
                t = ci * LOAD_CHUNK + j
                et = e_tile(t)
                # plain one-hot (tensor_scalar is ~10x slower than broadcast
                # tensor_tensor — use TT against a bf16 iota)
                oht = work.tile([P, C], BF16, tag="oht")
                nc.vector.tensor_tensor(
                    out=oht[:], in0=iota_row[:],
                    in1=lab_f[:, t : t + 1].to_broadcast([P, C]),
                    op=ALU.is_equal,
                )
                # r-scaled one-hot for the normalized class sums; alternate
                # the scaling between ACT and DVE to balance engine load
                osc = work.tile([P, C], BF16, tag="osc")
                if t % 2 == 0:
                    nc.scalar.activation(
                        osc[:], oht[:], ACTF.Copy, scale=r_all[:, t : t + 1]
                    )
                else:
                    nc.vector.tensor_tensor(
                        out=osc[:], in0=oht[:],
                        in1=r_bf[:, t : t + 1].to_broadcast([P, C]),
                        op=ALU.mult,
                    )
                first, last = t == 0, t == T - 1
                nc.tensor.matmul(
                    sums_ps0[:], osc[:, 0:P], et, start=first, stop=last
                )
                nc.tensor.matmul(
                    sums_ps1[:], osc[:, P:C], et, start=first, stop=last
                )
                nc.tensor.matmul(
                    cnt_ps[:], ones_col_bf[:], oht[:], start=first, stop=last
                )

        # ---- all-reduce sums + counts ----------------------------------
        sums_sb = [mid.tile([P, D], F32, tag=f"ssb{h}", name=f"ssb{h}") for h in range(2)]
        nc.vector.tensor_copy(out=sums_sb[0][:], in_=sums_ps0[:])
        nc.vector.tensor_copy(out=sums_sb[1][:], in_=sums_ps1[:])
        cnt_row = mid.tile([1, D], F32, tag="cntrow")
        nc.vector.memset(cnt_row[:], 0.0)
        nc.vector.tensor_copy(out=cnt_row[:, 0:C], in_=cnt_ps[:])

        nc.sync.dma_start(out=ar1_in.ap()[0:P, :], in_=sums_sb[0][:])
        nc.sync.dma_start(out=ar1_in.ap()[P:C, :], in_=sums_sb[1][:])
        nc.sync.dma_start(out=ar1_in.ap()[C : C + 1, :], in_=cnt_row[:])

        nc.gpsimd.collective_compute(
            "AllReduce", ALU.add, replica_groups=groups,
            ins=[ar1_in.ap()], outs=[ar1_out.ap()],
        )

        # global sums overwrite the local-sum tiles (same slots, AR is done)
        gsums = [mid.tile([P, D], F32, tag=f"ssb{h}", name=f"gs{h}") for h in range(2)]
        nc.sync.dma_start(out=gsums[0][:], in_=ar1_out.ap()[0:P, :])
        nc.sync.dma_start(out=gsums[1][:], in_=ar1_out.ap()[P:C, :])
        gcnt_row = mid.tile([1, C], F32, tag="cntrow")
        nc.sync.dma_start(out=gcnt_row[:], in_=ar1_out.ap()[C : C + 1, 0:C])

        # ---- centroids: cent = sums / max(||sums||, eps) ---------------
        cent = []
        for h in range(2):
            s2 = sq.tile([P, D], F32, tag="sq")  # scratch for the squares
            cn2 = mid.tile([P, 1], F32, tag=f"cn{h}")
            nc.scalar.activation(
                s2[:], gsums[h][:], ACTF.Square, accum_out=cn2[:]
            )
            nc.scalar.activation(cn2[:], cn2[:], ACTF.Sqrt)
            nc.vector.tensor_scalar(
                out=cn2[:], in0=cn2[:], scalar1=EPS, scalar2=None, op0=ALU.max
            )
            nc.vector.reciprocal(cn2[:], cn2[:])
            ch = mid.tile([P, D], F32, tag=f"cent{h}")
            nc.vector.tensor_scalar(
                out=ch[:], in0=gsums[h][:], scalar1=cn2[:], scalar2=None,
                op0=ALU.mult,
            )
            cent.append(ch)

        # ---- presence masks, counts columns, w -------------------------
        negmask_r = mid.tile([1, C], F32, tag="negm")
        nc.vector.tensor_scalar(
            out=negmask_r[:], in0=gcnt_row[:], scalar1=0.5, scalar2=float(NEG),
            op0=ALU.is_lt, op1=ALU.mult,
        )
        present_r = mid.tile([1, C], F32, tag="pres")
        nc.vector.tensor_scalar(
            out=present_r[:], in0=gcnt_row[:], scalar1=0.5, scalar2=None,
            op0=ALU.is_ge,
        )
        npres = mid.tile([1, 1], F32, tag="npres")
        nc.vector.reduce_sum(npres[:], present_r[:], axis=mybir.AxisListType.X)
        nc.vector.tensor_scalar(
            out=npres[:], in0=npres[:], scalar1=1.0, scalar2=None, op0=ALU.max
        )
        inv_np = mid.tile([1, 1], F32, tag="invnp")
        nc.vector.reciprocal(inv_np[:], npres[:])

        wcol = []
        for h in range(2):
            ccol_ps = psmid.tile([P, 1], F32, tag="m")
            nc.tensor.matmul(
                ccol_ps[:], gcnt_row[:, h * P : (h + 1) * P], ones_row[:, 0:1]
            )
            wc = mid.tile([P, 1], F32, tag=f"w{h}")
            nc.vector.tensor_scalar(
                out=wc[:], in0=ccol_ps[:], scalar1=1.0, scalar2=None, op0=ALU.max
            )
            nc.vector.reciprocal(wc[:], wc[:])
            wcol.append(wc)

        # ---- centroid similarity G = cent @ cent.T ---------------------
        centT = [mid.tile([P, C], F32, tag=f"ct{k}", name=f"ct{k}") for k in range(4)]
        for h in range(2):
            for k in range(4):
                tp = psmid.tile([P, P], F32, tag="m")
                nc.tensor.transpose(
                    tp[:], cent[h][:, k * P : (k + 1) * P], ident[:]
                )
                nc.vector.tensor_copy(
                    out=centT[k][:, h * P : (h + 1) * P], in_=tp[:]
                )

        g_sb = []
        for h in range(2):
            gp = psmid.tile([P, C], F32, tag="m")
            for k in range(4):
                nc.tensor.matmul(
                    gp[:], centT[k][:, h * P : (h + 1) * P], centT[k][:],
                    start=(k == 0), stop=(k == 3),
                )
            gs = mid.tile([P, C], F32, tag=f"g{h}")
            nc.vector.tensor_copy(out=gs[:], in_=gp[:])
            # mask the diagonal (self-similarity): keep where col - row != 0
            nc.gpsimd.affine_select(
                out=gs[:], in_=gs[:], compare_op=ALU.not_equal, fill=NEG,
                base=-h * P, pattern=[[1, C]], channel_multiplier=-1,
            )
            g_sb.append(gs)

        # add -1e30 to columns of empty classes (broadcast the row via PE)
        maskp = psmid.tile([P, C], F32, tag="m")
        nc.tensor.matmul(maskp[:], ones_row[:], negmask_r[:])
        for h in range(2):
            nc.vector.tensor_tensor(
                out=g_sb[h][:], in0=g_sb[h][:], in1=maskp[:], op=ALU.add
            )

        # ---- nearest-centroid one-hot (argmax by equality) -------------
        nst = [mid.tile([P, C], F32, tag=f"nst{k}", name=f"nst{k}") for k in range(2)]
        for h in range(2):
            mx = mid.tile([P, 1], F32, tag=f"mx{h}")
            nc.vector.reduce_max(mx[:], g_sb[h][:], axis=mybir.AxisListType.X)
            ns = mid.tile([P, C], F32, tag=f"ns{h}")
            nc.vector.tensor_scalar(
                out=ns[:], in0=g_sb[h][:], scalar1=mx[:], scalar2=None,
                op0=ALU.is_equal,
            )
            for k in range(2):
                tp = psmid.tile([P, P], F32, tag="m")
                nc.tensor.transpose(tp[:], ns[:, k * P : (k + 1) * P], ident[:])
                nc.vector.tensor_copy(
                    out=nst[k][:, h * P : (h + 1) * P], in_=tp[:]
                )

        # ---- u = w*(cent_near - cent), b = w*margin; write table -------
        for h in range(2):
            cnear = psmid.tile([P, D], F32, tag="m")
            for k in range(2):
                nc.tensor.matmul(
                    cnear[:], nst[k][:, h * P : (h + 1) * P], cent[k][:],
                    start=(k == 0), stop=(k == 1),
                )
            # k*u in fp32, then round to the bf16 row; b and k^2|u|^2 are
            # stored as bf16 hi+lo pairs to keep fp32-level precision
            uf = mid.tile([P, D], F32, tag="uf")
            nc.vector.tensor_tensor(
                out=uf[:], in0=cnear[:], in1=cent[h][:], op=ALU.subtract
            )
            nc.vector.tensor_scalar(
                out=uf[:], in0=uf[:], scalar1=wcol[h][:],
                scalar2=KAPPA, op0=ALU.mult, op1=ALU.mult,
            )
            tbl_sb = mid.tile([P, TBL_B], BF16, tag=f"tb{h}")
            nc.vector.tensor_copy(out=tbl_sb[:, 0:D], in_=uf[:])
            bcol = mid.tile([P, 1], F32, tag=f"bc{h}")
            nc.vector.tensor_scalar(
                out=bcol[:], in0=wcol[h][:], scalar1=MARGIN, scalar2=None,
                op0=ALU.mult,
            )
            u2col = mid.tile([P, 1], F32, tag=f"u2{h}")
            squ = sq.tile([P, D], F32, tag="sq")
            nc.scalar.activation(
                squ[:], tbl_sb[:, 0:D], ACTF.Square, accum_out=u2col[:]
            )
            lo = mid.tile([P, 1], F32, tag=f"lo{h}")
            nc.vector.tensor_copy(out=tbl_sb[:, D : D + 1], in_=bcol[:])
            nc.vector.tensor_copy(out=lo[:], in_=tbl_sb[:, D : D + 1])
            nc.vector.tensor_tensor(out=lo[:], in0=bcol[:], in1=lo[:],
                                    op=ALU.subtract)
            nc.vector.tensor_copy(out=tbl_sb[:, D + 1 : D + 2], in_=lo[:])
            nc.vector.tensor_copy(out=tbl_sb[:, D + 2 : D + 3], in_=u2col[:])
            nc.vector.tensor_copy(out=lo[:], in_=tbl_sb[:, D + 2 : D + 3])
            nc.vector.tensor_tensor(out=lo[:], in0=u2col[:], in1=lo[:],
                                    op=ALU.subtract)
            nc.vector.tensor_copy(out=tbl_sb[:, D + 3 : D + 4], in_=lo[:])
            nc.vector.memset(tbl_sb[:, D + 4 : TBL_B], 0.0)
            nc.sync.dma_start(out=table.ap()[h * P : (h + 1) * P, :], in_=tbl_sb[:])

        # ---- pass 2: gather (k*u, b, k^2|u|^2) by label; dot via the ----
        # difference of squares:  e.u = (|e + k*u|^2 - |e|^2 - k^2|u|^2)/2k.
        # (tensor_tensor_reduce is broken on HW; multi-index indirect
        # gathers too — one [P,1]-offset gather per 128-sample tile.)
        q_all = const.tile([P, T], F32)
        bu_all = const.tile([P, T, 4], F32)
        tiles_per_g = GCHUNK // P
        for gc in range(T // tiles_per_g):
            g_t = gat.tile([P, tiles_per_g, TBL_B], BF16, tag="g", name=f"g{gc}")
            nc.gpsimd.dma_gather(
                out_ap=g_t[:], in_ap=table.ap(),
                idxs_ap=lab16_sb[:, gc * (GCHUNK // 16) : (gc + 1) * (GCHUNK // 16)],
                num_idxs=GCHUNK, num_idxs_reg=GCHUNK, elem_size=TBL_B,
            )
            nc.vector.tensor_copy(
                out=bu_all[:, gc * tiles_per_g : (gc + 1) * tiles_per_g, :],
                in_=g_t[:, :, D : D + 4],
            )
            for j in range(tiles_per_g):
                t = gc * tiles_per_g + j
                s_t = sq.tile([P, D], F32, tag="pr")
                nc.vector.tensor_tensor(
                    out=s_t[:], in0=e_tile(t), in1=g_t[:, j, 0:D], op=ALU.add
                )
                sq2 = sq.tile([P, D], F32, tag="sq")
                nc.scalar.activation(
                    sq2[:], s_t[:], ACTF.Square, accum_out=q_all[:, t : t + 1]
                )


        # pre = (q - |e|^2 - k^2|u|^2) * (r / 2k) + b ;  term = relu(pre)
        r2 = const.tile([P, T], F32)
        nc.vector.tensor_scalar(
            out=r2[:], in0=r_all[:], scalar1=1.0 / (2.0 * KAPPA), scalar2=None,
            op0=ALU.mult,
        )
        pre_all = const.tile([P, T], F32)
        nc.vector.tensor_tensor(
            out=pre_all[:], in0=q_all[:], in1=norm2[:], op=ALU.subtract
        )
        nc.vector.tensor_tensor(
            out=pre_all[:], in0=pre_all[:], in1=bu_all[:, :, 2], op=ALU.subtract
        )
        nc.vector.tensor_tensor(
            out=pre_all[:], in0=pre_all[:], in1=bu_all[:, :, 3], op=ALU.subtract
        )
        nc.vector.tensor_tensor(
            out=pre_all[:], in0=pre_all[:], in1=r2[:], op=ALU.mult
        )
        nc.vector.tensor_tensor(
            out=pre_all[:], in0=pre_all[:], in1=bu_all[:, :, 0], op=ALU.add
        )
        nc.vector.tensor_tensor(
            out=pre_all[:], in0=pre_all[:], in1=bu_all[:, :, 1], op=ALU.add
        )
        con_all = const.tile([P, T], F32)
        nc.scalar.activation(con_all[:], pre_all[:], ACTF.Relu)

        tot_col = mid.tile([P, 1], F32, tag="tot")
        nc.vector.reduce_sum(tot_col[:], con_all[:], axis=mybir.AxisListType.X)
        tot_ps = psmid.tile([1, 1], F32, tag="m")
        nc.tensor.matmul(tot_ps[:], tot_col[:], ones_col[:])
        tot_sb = mid.tile([1, 8], F32, tag="totsb")
        nc.vector.memset(tot_sb[:], 0.0)
        nc.vector.tensor_copy(out=tot_sb[:, 0:1], in_=tot_ps[:])
        nc.sync.dma_start(out=ar2_in.ap()[:], in_=tot_sb[:])
        nc.gpsimd.collective_compute(
            "AllReduce", ALU.add, replica_groups=groups,
            ins=[ar2_in.ap()], outs=[ar2_out.ap()],
        )
        gtot = mid.tile([1, 8], F32, tag="gtot")
        nc.sync.dma_start(out=gtot[:], in_=ar2_out.ap()[:])
        loss_sb = mid.tile([1, 1], F32, tag="loss")
        nc.vector.tensor_tensor(
            out=loss_sb[:], in0=gtot[:, 0:1], in1=inv_np[:], op=ALU.mult
        )
        nc.sync.dma_start(out=loss_out.ap()[:], in_=loss_sb[:])

    nc.compile()
    return nc


_NC = None


def _get_nc():
    global _NC
    if _NC is None:
        _NC = _build()
    return _NC


def build_in_maps(emb: np.ndarray, lab: np.ndarray) -> list[dict]:
    """Shard full inputs across the 8 cores (batch-dim data parallel)."""
    in_maps = []
    for c in range(N_CORES):
        sl = slice(c * B_LOC, (c + 1) * B_LOC)
        lab_c = lab[sl]
        lab_2d = np.ascontiguousarray(lab_c.reshape(T, P).T)  # [P, T]
        wrapped = lab_c.astype(np.int16).reshape(B_LOC // 16, 16).T
        lab16_2d = np.ascontiguousarray(np.tile(wrapped, (P // 16, 1)))
        in_maps.append({"emb": emb[sl], "lab": lab_2d, "lab16": lab16_2d})
    return in_maps


def kernel(embeddings: np.ndarray, labels: np.ndarray) -> np.ndarray:
    emb = np.ascontiguousarray(np.asarray(embeddings, dtype=np.float32))
    lab = np.asarray(labels).astype(np.int32)
    assert emb.shape == (B_FULL, D) and lab.shape == (B_FULL,)

    nc = _get_nc()
    in_maps = build_in_maps(emb, lab)
    res = run_bass_kernel_spmd(nc, in_maps, core_ids=list(range(N_CORES)))
    loss = res.results[0]["loss"]
    return np.asarray(loss, dtype=np.float32).reshape(())


if __name__ == "__main__":
    rng = np.random.default_rng(0)
    e = rng.standard_normal((B_FULL, D), dtype=np.float32)
    l = rng.integers(0, C, size=(B_FULL,)).astype(np.int32)
    print(kernel(embeddings=e, labels=l))



# revision 6
# speedup vs baseline: 1.1657x; 1.1657x over previous
"""Centroid triplet loss on 8 Trainium2 NeuronCores (Bass/Tile) — v2.

Data-parallel over the batch: each of the 8 cores gets 8192 of the 65536
samples.  Per-class embedding sums (counts folded in as an extra column)
are all-reduced in two sample-halves (the first overlaps the second half
of pass 1); each core then computes its local triplet terms and a final
all-reduce produces the scalar loss.

Math restructure (equivalent to the reference):
    term_i = relu(margin + e_hat_i . (cent[nearest[l_i]] - cent[l_i]))
    loss   = sum_c w_c * segsum_c(term) / n_present,   w_c = 1/max(count_c, 1)
Key points vs v1:
  * counts ride inside the sums matmul: e is augmented with a ||e|| column;
    with the r-scaled one-hot stationary that column accumulates
    sum_i r_i*||e_i|| = count_c.  No separate counts matmul.
  * one-hot and its r-scaled variant are built in two fused DVE ops (the
    scaled one via scalar_tensor_tensor); the one-hot is kept in SBUF for
    pass 2's segment-sum.
  * pass 2 gathers raw (unweighted) delta-centroid rows (1024B each) and
    computes each tile's dot in ONE fused scalar_tensor_tensor:
    dot = sum((e*r) * dc).  relu(dot + margin) is one ACT op; the
    per-class segment sum is a 1-column matmul accumulating into a single
    PSUM row; per-class weights w_c are applied once at the end.
"""

import sys

for _p in ("/opt/trn_rl_repo",):
    if _p not in sys.path:
        sys.path.insert(0, _p)

from contextlib import ExitStack

import numpy as np

from concourse import bacc, bass, mybir, tile
from concourse.bass_utils import run_bass_kernel_spmd
from concourse.masks import make_identity

F32 = mybir.dt.float32
BF16 = mybir.dt.bfloat16
I32 = mybir.dt.int32
I16 = mybir.dt.int16
ALU = mybir.AluOpType
ACTF = mybir.ActivationFunctionType

N_CORES = 8
B_FULL = 65536
D = 512
C = 256
MARGIN = 0.3
EPS = 1e-12

P = 128                      # SBUF partitions
B_LOC = B_FULL // N_CORES    # 8192 samples per core
T = B_LOC // P               # 64 sample tiles of 128
LOAD_CHUNK = 8               # tiles per embedding-load DMA
DA = D + 8                   # e row: e[0:512], ||e|| at col 512, zero pad
DH = DA // 2                 # 260 fp32 = 1040B: fits a PSUM bank
GCHUNK = 1024                # indices per dma_gather call (8 sample tiles)
NEG = -1e30


def _build():
    nc = bacc.Bacc(
        "TRN2",
        target_bir_lowering=False,
        debug=False,
        enable_asserts=False,
        num_devices=N_CORES,
    )

    emb = nc.dram_tensor("emb", [B_LOC, D], F32, kind="ExternalInput")
    lab = nc.dram_tensor("lab", [P, T], I32, kind="ExternalInput")
    # labels in dma_gather's wrapped-int16 layout
    lab16 = nc.dram_tensor("lab16", [P, B_LOC // 16], I16, kind="ExternalInput")
    loss_out = nc.dram_tensor("loss", [1, 1], F32, kind="ExternalOutput")

    # Internal HBM scratch.  Two half-batch AR buffers: rows are classes,
    # cols 0:512 per-class sums, col 512 per-class counts, rest zero pad.
    ar_in = [nc.dram_tensor(f"ar{h}_in", [C, DA], F32) for h in range(2)]
    ar_out = [
        nc.dram_tensor(f"ar{h}_out", [C, DA], F32, addr_space="Shared")
        for h in range(2)
    ]
    table = nc.dram_tensor("tbl", [C, D], BF16)
    ar2_in = nc.dram_tensor("ar2_in", [1, 8], F32)
    ar2_out = nc.dram_tensor("ar2_out", [1, 8], F32, addr_space="Shared")

    groups = [list(range(N_CORES))]

    with tile.TileContext(nc) as tc, ExitStack() as ctx:
        const = ctx.enter_context(tc.tile_pool(name="const", bufs=1))
        big = ctx.enter_context(tc.tile_pool(name="big", bufs=1))
        work = ctx.enter_context(tc.tile_pool(name="work", bufs=3))
        sq = ctx.enter_context(tc.tile_pool(name="sq", bufs=2))
        gat = ctx.enter_context(tc.tile_pool(name="gat", bufs=3))
        mid = ctx.enter_context(tc.tile_pool(name="mid", bufs=1))

        # ---- constants -------------------------------------------------
        ident = const.tile([P, P], F32)
        make_identity(nc, ident[:])
        ident_bf = const.tile([P, P], BF16)
        nc.vector.tensor_copy(out=ident_bf[:], in_=ident[:])
        iota_row = const.tile([P, C], BF16)
        nc.gpsimd.iota(
            iota_row[:], pattern=[[1, C]], base=0, channel_multiplier=0,
            allow_small_or_imprecise_dtypes=True,
        )
        ones_row = const.tile([1, P], F32)
        nc.gpsimd.memset(ones_row[:], 1.0)
        marg_col = const.tile([P, 1], F32)
        nc.gpsimd.memset(marg_col[:], MARGIN)

        lab_sb = const.tile([P, T], I32)
        nc.sync.dma_start(out=lab_sb[:], in_=lab.ap())
        lab_f = const.tile([P, T], BF16)
        nc.vector.tensor_copy(out=lab_f[:], in_=lab_sb[:])
        lab16_sb = const.tile([P, B_LOC // 16], I16)
        nc.sync.dma_start(out=lab16_sb[:], in_=lab16.ap())

        # ---- pass 1: load embeddings, norms, class sums (+counts) ------
        e_chunks = []
        emb_v = emb.ap().rearrange("(t p) d -> p t d", p=P)
        for ci in range(T // LOAD_CHUNK):
            ec = big.tile([P, LOAD_CHUNK, DA], BF16, tag=f"e{ci}")
            e_chunks.append(ec)
            # zero the aug cols; DMA fills 0:512; ||e|| lands in col 512
            nc.vector.memset(ec[:, :, D:DA], 0.0)
            sl = slice(ci * LOAD_CHUNK, (ci + 1) * LOAD_CHUNK)
            nc.gpsimd.dma_start(out=ec[:, :, 0:D], in_=emb_v[:, sl, :])

        norm2 = const.tile([P, T], F32)
        norm = const.tile([P, T], F32)
        r_all = const.tile([P, T], F32)
        r_bf = const.tile([P, T], BF16)
        oht_tiles = []

        def e_tile(t):
            return e_chunks[t // LOAD_CHUNK][:, t % LOAD_CHUNK, :]

        with tc.tile_pool(name="psacc", bufs=1, space="PSUM") as psacc:
            # [half][class_chunk][d_half] accumulators, one PSUM bank each
            sums_ps = [
                [
                    [
                        psacc.tile([P, DH], F32, tag=f"s{h}{cc}{dh}", name=f"s{h}{cc}{dh}")
                        for dh in range(2)
                    ]
                    for cc in range(2)
                ]
                for h in range(2)
            ]

            for ci in range(T // LOAD_CHUNK):
                csl = slice(ci * LOAD_CHUNK, (ci + 1) * LOAD_CHUNK)
                for j in range(LOAD_CHUNK):
                    t = ci * LOAD_CHUNK + j
                    sq_t = sq.tile([P, D], F32, tag="sq")
                    nc.scalar.activation(
                        sq_t[:], e_tile(t)[:, 0:D], ACTF.Square,
                        accum_out=norm2[:, t : t + 1],
                    )
                # batched per-chunk norm -> r; ||e|| into the aug column
                nc.scalar.activation(norm[:, csl], norm2[:, csl], ACTF.Sqrt)
                nc.vector.reciprocal(r_all[:, csl], norm[:, csl])
                nc.vector.tensor_copy(out=r_bf[:, csl], in_=r_all[:, csl])
                nc.vector.tensor_copy(
                    out=e_chunks[ci][:, :, D : D + 1], in_=norm[:, csl]
                )

                for j in range(LOAD_CHUNK):
                    t = ci * LOAD_CHUNK + j
                    h = t // (T // 2)
                    et = e_tile(t)
                    oht = big.tile([P, C], BF16, tag=f"ohtk{t}")
                    oht_tiles.append(oht)
                    nc.vector.tensor_tensor(
                        out=oht[:], in0=iota_row[:],
                        in1=lab_f[:, t : t + 1].to_broadcast([P, C]),
                        op=ALU.is_equal,
                    )
                    # r-scaled one-hot in one fused op
                    osc = work.tile([P, C], BF16, tag="osc")
                    nc.vector.scalar_tensor_tensor(
                        out=osc[:], in0=iota_row[:],
                        scalar=lab_f[:, t : t + 1],
                        in1=r_bf[:, t : t + 1].to_broadcast([P, C]),
                        op0=ALU.is_equal, op1=ALU.mult,
                    )
                    first = t % (T // 2) == 0
                    last = t % (T // 2) == (T // 2) - 1
                    for cc in range(2):
                        for dh in range(2):
                            nc.tensor.matmul(
                                sums_ps[h][cc][dh][:],
                                osc[:, cc * P : (cc + 1) * P],
                                et[:, dh * DH : (dh + 1) * DH],
                                start=first, stop=last,
                            )

                # after each half: psum -> sbuf -> HBM -> AllReduce
                if ci in (T // (2 * LOAD_CHUNK) - 1, T // LOAD_CHUNK - 1):
                    h = 0 if ci == T // (2 * LOAD_CHUNK) - 1 else 1
                    half_sb = [
                        mid.tile([P, DA], F32, tag=f"hsb{h}{cc}", name=f"hsb{h}{cc}")
                        for cc in range(2)
                    ]
                    for cc in range(2):
                        for dh in range(2):
                            nc.vector.tensor_copy(
                                out=half_sb[cc][:, dh * DH : (dh + 1) * DH],
                                in_=sums_ps[h][cc][dh][:],
                            )
                        nc.sync.dma_start(
                            out=ar_in[h].ap()[cc * P : (cc + 1) * P, :],
                            in_=half_sb[cc][:],
                        )
                    nc.gpsimd.collective_compute(
                        "AllReduce", ALU.add, replica_groups=groups,
                        ins=[ar_in[h].ap()], outs=[ar_out[h].ap()],
                    )

        psloss = ctx.enter_context(
            tc.tile_pool(name="psloss", bufs=1, space="PSUM")
        )
        psmid = ctx.enter_context(
            tc.tile_pool(name="psmid", bufs=3, space="PSUM")
        )

        # ---- global sums: add the two AR halves ------------------------
        gsums = [mid.tile([P, DA], F32, tag=f"gs{cc}", name=f"gs{cc}") for cc in range(2)]
        half_in = [
            [mid.tile([P, DA], F32, tag=f"hin{h}{cc}", name=f"hin{h}{cc}") for cc in range(2)]
            for h in range(2)
        ]
        for h in range(2):
            for cc in range(2):
                nc.sync.dma_start(
                    out=half_in[h][cc][:],
                    in_=ar_out[h].ap()[cc * P : (cc + 1) * P, :],
                )
        for cc in range(2):
            nc.vector.tensor_tensor(
                out=gsums[cc][:], in0=half_in[0][cc][:],
                in1=half_in[1][cc][:], op=ALU.add,
            )

        # ---- centroids: cent = sums / max(||sums||, eps) ---------------
        cent = []
        centb = []
        for cc in range(2):
            s2 = sq.tile([P, D], F32, tag="sq")
            cn2 = mid.tile([P, 1], F32, tag=f"cn{cc}")
            nc.vector.scalar_tensor_tensor(
                out=s2[:], in0=gsums[cc][:, 0:D], scalar=1.0,
                in1=gsums[cc][:, 0:D], op0=ALU.mult, op1=ALU.mult,
                accum_out=cn2[:],
            )
            nc.scalar.activation(cn2[:], cn2[:], ACTF.Sqrt)
            nc.vector.tensor_scalar(
                out=cn2[:], in0=cn2[:], scalar1=EPS, scalar2=None, op0=ALU.max
            )
            nc.vector.reciprocal(cn2[:], cn2[:])
            ch = mid.tile([P, D], F32, tag=f"cent{cc}")
            nc.vector.tensor_scalar(
                out=ch[:], in0=gsums[cc][:, 0:D], scalar1=cn2[:],
                scalar2=None, op0=ALU.mult,
            )
            cent.append(ch)
            cb = mid.tile([P, D], BF16, tag=f"centb{cc}")
            nc.vector.tensor_copy(out=cb[:], in_=ch[:])
            centb.append(cb)

        # ---- counts row, presence, weights -----------------------------
        gcnt_row = mid.tile([1, C], F32, tag="cntrow")
        for cc in range(2):
            tp = psmid.tile([1, P], F32, tag="mrow")
            nc.tensor.transpose(tp[:], gsums[cc][:, D : D + 1], ident[:])
            nc.vector.tensor_copy(
                out=gcnt_row[:, cc * P : (cc + 1) * P], in_=tp[:]
            )
        negmask_r = mid.tile([1, C], F32, tag="negm")
        nc.vector.tensor_scalar(
            out=negmask_r[:], in0=gcnt_row[:], scalar1=0.5,
            scalar2=float(NEG), op0=ALU.is_lt, op1=ALU.mult,
        )
        present_r = mid.tile([1, C], F32, tag="pres")
        nc.vector.tensor_scalar(
            out=present_r[:], in0=gcnt_row[:], scalar1=0.5, scalar2=None,
            op0=ALU.is_ge,
        )
        npres = mid.tile([1, 1], F32, tag="npres")
        nc.vector.reduce_sum(npres[:], present_r[:], axis=mybir.AxisListType.X)
        nc.vector.tensor_scalar(
            out=npres[:], in0=npres[:], scalar1=1.0, scalar2=None, op0=ALU.max
        )
        inv_np = mid.tile([1, 1], F32, tag="invnp")
        nc.vector.reciprocal(inv_np[:], npres[:])
        wrow = mid.tile([1, C], F32, tag="wrow")
        nc.vector.tensor_scalar(
            out=wrow[:], in0=gcnt_row[:], scalar1=1.0, scalar2=None,
            op0=ALU.max,
        )
        nc.vector.reciprocal(wrow[:], wrow[:])

        # ---- centroid similarity G = cent @ cent.T (bf16) --------------
        centT = [mid.tile([P, C], BF16, tag=f"ct{k}", name=f"ct{k}") for k in range(4)]
        for cc in range(2):
            for k in range(4):
                tp = psmid.tile([P, P], F32, tag="m")
                nc.tensor.transpose(
                    tp[:], cent[cc][:, k * P : (k + 1) * P], ident[:]
                )
                nc.vector.tensor_copy(
                    out=centT[k][:, cc * P : (cc + 1) * P], in_=tp[:]
                )

        g_sb = []
        for cc in range(2):
            gp = psmid.tile([P, C], F32, tag="m")
            for k in range(4):
                nc.tensor.matmul(
                    gp[:], centT[k][:, cc * P : (cc + 1) * P], centT[k][:],
                    start=(k == 0), stop=(k == 3),
                )
            gs = mid.tile([P, C], F32, tag=f"g{cc}")
            nc.vector.tensor_copy(out=gs[:], in_=gp[:])
            # mask the diagonal (self-similarity)
            nc.gpsimd.affine_select(
                out=gs[:], in_=gs[:], compare_op=ALU.not_equal, fill=NEG,
                base=-cc * P, pattern=[[1, C]], channel_multiplier=-1,
            )
            g_sb.append(gs)

        # empty classes can't be negatives
        maskp = psmid.tile([P, C], F32, tag="m")
        nc.tensor.matmul(maskp[:], ones_row[:], negmask_r[:])
        for cc in range(2):
            nc.vector.tensor_tensor(
                out=g_sb[cc][:], in0=g_sb[cc][:], in1=maskp[:], op=ALU.add
            )

        # ---- nearest-centroid one-hot (argmax by equality, tie-averaged)
        nst = [mid.tile([P, C], BF16, tag=f"nst{k}", name=f"nst{k}") for k in range(2)]
        for cc in range(2):
            mx = mid.tile([P, 1], F32, tag=f"mx{cc}")
            nc.vector.reduce_max(mx[:], g_sb[cc][:], axis=mybir.AxisListType.X)
            ns = mid.tile([P, C], F32, tag=f"ns{cc}")
            nc.vector.tensor_scalar(
                out=ns[:], in0=g_sb[cc][:], scalar1=mx[:], scalar2=None,
                op0=ALU.is_equal,
            )
            # bf16 G can tie: average the tied centroids
            tn = mid.tile([P, 1], F32, tag=f"tn{cc}")
            nc.vector.reduce_sum(tn[:], ns[:], axis=mybir.AxisListType.X)
            nc.vector.reciprocal(tn[:], tn[:])
            nc.vector.tensor_scalar(
                out=ns[:], in0=ns[:], scalar1=tn[:], scalar2=None,
                op0=ALU.mult,
            )
            for k in range(2):
                tp = psmid.tile([P, P], F32, tag="m")
                nc.tensor.transpose(
                    tp[:], ns[:, k * P : (k + 1) * P], ident[:]
                )
                nc.vector.tensor_copy(
                    out=nst[k][:, cc * P : (cc + 1) * P], in_=tp[:]
                )

        # ---- dc = cent_near - cent -> bf16 table in HBM ----------------
        for cc in range(2):
            cnear = psmid.tile([P, D], F32, tag="m")
            for k in range(2):
                nc.tensor.matmul(
                    cnear[:], nst[k][:, cc * P : (cc + 1) * P], centb[k][:],
                    start=(k == 0), stop=(k == 1),
                )
            dcb = mid.tile([P, D], BF16, tag=f"dc{cc}")
            nc.vector.tensor_tensor(
                out=dcb[:], in0=cnear[:], in1=cent[cc][:], op=ALU.subtract
            )
            nc.sync.dma_start(
                out=table.ap()[cc * P : (cc + 1) * P, :], in_=dcb[:]
            )

        # ---- pass 2: gather dc rows; fused dot; relu; segment-sum ------
        loss_ps = psloss.tile([1, C], F32, tag="lps")
        tiles_per_g = GCHUNK // P
        for gc in range(T // tiles_per_g):
            g_t = gat.tile([P, tiles_per_g, D], BF16, tag="g", name=f"g{gc}")
            nc.gpsimd.dma_gather(
                out_ap=g_t[:], in_ap=table.ap(),
                idxs_ap=lab16_sb[
                    :, gc * (GCHUNK // 16) : (gc + 1) * (GCHUNK // 16)
                ],
                num_idxs=GCHUNK, num_idxs_reg=GCHUNK, elem_size=D,
            )
            for j in range(tiles_per_g):
                t = gc * tiles_per_g + j
                s_t = sq.tile([P, D], F32, tag="pr")
                dot = work.tile([P, 1], F32, tag="dot")
                nc.vector.scalar_tensor_tensor(
                    out=s_t[:], in0=e_tile(t)[:, 0:D],
                    scalar=r_all[:, t : t + 1], in1=g_t[:, j, :],
                    op0=ALU.mult, op1=ALU.mult, accum_out=dot[:],
                )
                term = work.tile([P, 1], BF16, tag="term")
                nc.scalar.activation(term[:], dot[:], ACTF.Relu, bias=marg_col[:])
                nc.tensor.matmul(
                    loss_ps[:], term[:], oht_tiles[t][:],
                    start=(t == 0), stop=(t == T - 1),
                )

        # ---- loss = sum_c w_c * segsum_c / n_present; all-reduce -------
        tail_scr = mid.tile([1, C], F32, tag="tscr")
        partial = mid.tile([1, 1], F32, tag="part")
        nc.vector.scalar_tensor_tensor(
            out=tail_scr[:], in0=loss_ps[:], scalar=inv_np[:], in1=wrow[:],
            op0=ALU.mult, op1=ALU.mult, accum_out=partial[:],
        )
        tot_sb = mid.tile([1, 8], F32, tag="totsb")
        nc.vector.memset(tot_sb[:], 0.0)
        nc.vector.tensor_copy(out=tot_sb[:, 0:1], in_=partial[:])
        nc.sync.dma_start(out=ar2_in.ap()[:], in_=tot_sb[:])
        nc.gpsimd.collective_compute(
            "AllReduce", ALU.add, replica_groups=groups,
            ins=[ar2_in.ap()], outs=[ar2_out.ap()],
        )
        gtot = mid.tile([1, 8], F32, tag="gtot")
        nc.sync.dma_start(out=gtot[:], in_=ar2_out.ap()[:])
        loss_sb = mid.tile([1, 1], F32, tag="loss")
        nc.vector.tensor_copy(out=loss_sb[:], in_=gtot[:, 0:1])
        nc.sync.dma_start(out=loss_out.ap()[:], in_=loss_sb[:])

    nc.compile()
    return nc


_NC = None


def _get_nc():
    global _NC
    if _NC is None:
        _NC = _build()
    return _NC


def build_in_maps(emb: np.ndarray, lab: np.ndarray) -> list[dict]:
    """Shard full inputs across the 8 cores (batch-dim data parallel)."""
    in_maps = []
    for c in range(N_CORES):
        sl = slice(c * B_LOC, (c + 1) * B_LOC)
        lab_c = lab[sl]
        lab_2d = np.ascontiguousarray(lab_c.reshape(T, P).T)  # [P, T]
        wrapped = lab_c.astype(np.int16).reshape(B_LOC // 16, 16).T
        lab16_2d = np.ascontiguousarray(np.tile(wrapped, (P // 16, 1)))
        in_maps.append({"emb": emb[sl], "lab": lab_2d, "lab16": lab16_2d})
    return in_maps


def kernel(embeddings: np.ndarray, labels: np.ndarray) -> np.ndarray:
    emb = np.ascontiguousarray(np.asarray(embeddings, dtype=np.float32))
    lab = np.asarray(labels).astype(np.int32)
    assert emb.shape == (B_FULL, D) and lab.shape == (B_FULL,)

    nc = _get_nc()
    in_maps = build_in_maps(emb, lab)
    res = run_bass_kernel_spmd(nc, in_maps, core_ids=list(range(N_CORES)))
    loss = res.results[0]["loss"]
    return np.asarray(loss, dtype=np.float32).reshape(())


if __name__ == "__main__":
    rng = np.random.default_rng(0)
    e = rng.standard_normal((B_FULL, D), dtype=np.float32)
    l = rng.integers(0, C, size=(B_FULL,)).astype(np.int32)
    print(kernel(embeddings=e, labels=l))
